# revision 2
# baseline (speedup 1.0000x reference)
"""Multi-head attention Trainium2 kernel (v2 — pipelined single-stream).

Problem: B=4, S=2048, D_MODEL=1024, H=16 heads, d_k=d_v=64.

Sharding (8 cores, no collectives): core c handles batch b=c//2 and head
group g=c%2 (8 heads). Host sums the two head-group partials per batch and
adds the folded biases (bk drops out of softmax; bv/bo fold into bo_eff).

Engine budget per core: PE ~276us (matmuls), ACT ~255us (exp of 8*2048^2
scores, the only thing ScalarE does), DVE ~150us (psum->sbuf copies +
softmax normalization). The kernel emits one interleaved stream of
(pair, q-chunk) units so PE and ACT overlap near-fully.

Key device tricks:
 - scores^T per (pair, sc, qc): [s=128, q| two heads 512+512] psum tile via
   two concurrent K=64 row-tiled matmuls (partition halves 0-63 / 64-127).
 - exp on ScalarE only, FD=1024 per instruction.
 - ctx: K=128 (full s-chunk) matmuls, lhsT = vha[:, sc, h, 0:128] where
   cols 64:128 are ones -> psum rows 0:64 = ctx_u, rows 64:128 = Z
   replicated 64x (the PE columns were otherwise idle; N-streaming bound).
 - 1/Z on DVE WITHOUT `reciprocal` (measured ~6.4 cyc/elem): bitcast-seed
   Newton: r0 = bitcast(MAGIC - int_bits(Z)), r1 = (Z*r0 - 2)*r0 = -1/Z
   (max rel err 2.6e-3, mean -1.3e-3; sign and bias folded into Wo on
   host: wo_dev = -(1+1.29e-3) * Wo).
 - q/k/v pre-converted to bf16 on host; output partials returned as bf16.
"""

import contextlib

import numpy as np
import ml_dtypes

import concourse.bass as bass
import concourse.bacc as bacc
import concourse.mybir as mybir
import concourse.tile as tile
from concourse.bass import ts

BF16 = mybir.dt.bfloat16
F32 = mybir.dt.float32
I32 = mybir.dt.int32
Alu = mybir.AluOpType

D_MODEL, D_K, D_V, N_HEADS = 1024, 64, 64, 16
B, S = 4, 2048
N_CORES = 8
NH = 8              # heads per core
HD = NH * D_V       # 512
T = S
DC = 8              # D_MODEL / 128
SCN = 16            # s chunks of 128
QCN = 4             # q chunks of 512
HCN = 4             # head-pair chunks of 128 (= pairs)
QPC = 8             # q-proj token chunks of 256
EXP_BUFS = 18

MAGICF = float(0x7EF31999)   # Newton reciprocal seed constant
RECIP_CORR = 1.0 + 1.29e-3   # centers the Newton bias; folded into Wo


def build_nc(reps: int = 1, phases: str = "all"):
    nc = bacc.Bacc("TRN2", target_bir_lowering=False, debug=False)

    xq_d = nc.dram_tensor("xq_t", [128, DC, T], BF16, kind="ExternalInput")
    xk_d = nc.dram_tensor("xk_t", [128, DC, T], BF16, kind="ExternalInput")
    xv_d = nc.dram_tensor("xv_t", [128, DC, T], BF16, kind="ExternalInput")
    wq_d = nc.dram_tensor("wq", [128, DC, HD], BF16, kind="ExternalInput")
    wk_d = nc.dram_tensor("wk", [128, DC, HD], BF16, kind="ExternalInput")
    wv_d = nc.dram_tensor("wv", [128, DC, HD], BF16, kind="ExternalInput")
    wo_d = nc.dram_tensor("wo", [128, HCN, D_MODEL], BF16, kind="ExternalInput")
    bq_d = nc.dram_tensor("bq", [128, HCN], F32, kind="ExternalInput")
    out_d = nc.dram_tensor("out", [SCN, 128, D_MODEL], BF16, kind="ExternalOutput")

    with tile.TileContext(nc) as tc:
        def body():
            emit_body(nc, tc, xq_d, xk_d, xv_d, wq_d, wk_d, wv_d, wo_d, bq_d, out_d, phases)

        if reps == 1:
            body()
        else:
            with tc.For_i(0, reps, 1):
                body()
    nc.compile()
    return nc


def emit_body(nc, tc, xq_d, xk_d, xv_d, wq_d, wk_d, wv_d, wo_d, bq_d, out_d, phases="all"):
    ctx = contextlib.ExitStack()
    with ctx:
        # ---------------- persistent SBUF ----------------
        wpool = ctx.enter_context(tc.tile_pool(name="wpool", bufs=1))
        big = ctx.enter_context(tc.tile_pool(name="big", bufs=1))
        khp = ctx.enter_context(tc.tile_pool(name="khp", bufs=2))
        qhp = ctx.enter_context(tc.tile_pool(name="qhp", bufs=1))
        opool = ctx.enter_context(tc.tile_pool(name="opool", bufs=4))
        npool = ctx.enter_context(tc.tile_pool(name="npool", bufs=2))

        wq_sb = wpool.tile([128, DC, HD], BF16, tag="wq")
        wk_sb = wpool.tile([128, DC, HD], BF16, tag="wk")
        wv_sb = wpool.tile([128, DC, HD], BF16, tag="wv")
        wo_sb = wpool.tile([128, HCN, D_MODEL], BF16, tag="wo")
        bq_sb = wpool.tile([128, HCN], F32, tag="bq")
        xk_sb = big.tile([128, DC, T], BF16, tag="xk")

        nc.sync.dma_start(wk_sb[:], wk_d[:])
        for tc_i in range(4):
            nc.sync.dma_start(xk_sb[:, :, ts(tc_i, 512)], xk_d[:, :, ts(tc_i, 512)])
        nc.sync.dma_start(wq_sb[:], wq_d[:])
        nc.sync.dma_start(bq_sb[:], bq_d[:])
        nc.sync.dma_start(wv_sb[:], wv_d[:])
        nc.sync.dma_start(wo_sb[:], wo_d[:])

        qhT = qhp.tile([128, HCN, T], BF16, tag="qhT")   # [dk of 2 heads, pair, t]
        ctxT = big.tile([128, HCN, T], BF16, tag="ctxT")
        vha = big.tile([128, SCN, NH, 128], BF16, tag="vha")  # [s%128, sc, h, dv|ones]
        nc.vector.memset(vha[:, :, :, D_V:128], 1.0)

        khT = {}  # pair -> ring tile [128, T]

        with (
            tc.tile_pool(name="xst", bufs=2) as xst,
            tc.tile_pool(name="vst", bufs=2) as vst,
            tc.tile_pool(name="expool", bufs=EXP_BUFS) as expool,
            tc.tile_pool(name="sp", bufs=2, space="PSUM") as sp,
            tc.tile_pool(name="cp", bufs=4, space="PSUM") as cp,
        ):
            exp_tiles = {}

            # ---------------- emit helpers ----------------
            def kproj_chunk(p, tc_i):
                # khT[p][:, tc-slice] = Wk[:, p-chunk].T @ xk chunk
                if p not in khT:
                    khT[p] = khp.tile([128, T], BF16, tag="khT", name=f"khT_{p}")
                pt = cp.tile([128, 512], F32, tag="cp", name=f"kp_{p}_{tc_i}")
                for dc in range(DC):
                    nc.tensor.matmul(
                        pt[:],
                        lhsT=wk_sb[:, dc, ts(p, 128)],
                        rhs=xk_sb[:, dc, ts(tc_i, 512)],
                        start=(dc == 0),
                        stop=(dc == DC - 1),
                    )
                nc.vector.tensor_copy(khT[p][:, ts(tc_i, 512)], pt[:])

            def qproj_chunk(p, qp):
                # qhT[:, p, 256-chunk qp] = Wq[:, p-chunk].T @ xq chunk + bq
                xb = xst.tile([128, DC, 256], BF16, tag="xst", name=f"xq_{p}_{qp}")
                nc.sync.dma_start(xb[:], xq_d[:, :, ts(qp, 256)])
                pt = cp.tile([128, 512], F32, tag="cp", name=f"qp_{p}_{qp}")
                for dc in range(DC):
                    nc.tensor.matmul(
                        pt[:, 0:256],
                        lhsT=wq_sb[:, dc, ts(p, 128)],
                        rhs=xb[:, dc, :],
                        start=(dc == 0),
                        stop=(dc == DC - 1),
                    )
                nc.vector.tensor_scalar_add(
                    qhT[:, p, ts(qp, 256)], pt[:, 0:256], bq_sb[:, p : p + 1]
                )

            def vproj_chunk(sc):
                # vha[:, sc, :, 0:64] = xv chunk.T @ Wv
                vx = vst.tile([128, DC, 128], BF16, tag="vst", name=f"xv_{sc}")
                nc.sync.dma_start(vx[:], xv_d[:, :, ts(sc, 128)])
                pv = cp.tile([128, 512], F32, tag="cp", name=f"vp_{sc}")
                for dc in range(DC):
                    nc.tensor.matmul(
                        pv[:],
                        lhsT=vx[:, dc, :],
                        rhs=wv_sb[:, dc, :],
                        start=(dc == 0),
                        stop=(dc == DC - 1),
                    )
                nc.vector.tensor_copy(
                    vha[:, sc, :, 0:D_V],
                    pv[:].rearrange("p (h d) -> p h d", d=D_V),
                )

            def scores_exp(p, qc):
                for sc in range(SCN):
                    s_ps = sp.tile([128, 1024], F32, tag="sp", name="s_ps")
                    for hl in range(2):
                        pb = hl * 64
                        nc.tensor.matmul(
                            s_ps[:, ts(hl, 512)],
                            lhsT=khT[p][pb : pb + 64, ts(sc, 128)],
                            rhs=qhT[pb : pb + 64, p, ts(qc, 512)],
                            start=True,
                            stop=True,
                        )
                    e = expool.tile([128, 1024], BF16, tag="exp", name="exp_t")
                    nc.scalar.activation(
                        e[:], s_ps[:], mybir.ActivationFunctionType.Exp, scale=0.125
                    )
                    exp_tiles[(p, qc, sc)] = e

            def ctx_unit(p, qc):
                # both heads of pair p for q-chunk qc; ones cols give Z rows
                c_ps = [
                    cp.tile([128, 512], F32, tag="cp", name=f"c_{hl}") for hl in range(2)
                ]
                for sc in range(SCN):
                    e = exp_tiles[(p, qc, sc)]
                    for hl in range(2):
                        nc.tensor.matmul(
                            c_ps[hl][:],
                            lhsT=vha[:, sc, 2 * p + hl, :],
                            rhs=e[:, ts(hl, 512)],
                            start=(sc == 0),
                            stop=(sc == SCN - 1),
                        )
                for sc in range(SCN):
                    del exp_tiles[(p, qc, sc)]
                for hl in range(2):
                    c = c_ps[hl]
                    zi = c[64:128, :].bitcast(I32)
                    r0f = npool.tile([64, 512], F32, tag="r0f", name="r0f")
                    nc.vector.tensor_scalar(
                        r0f[:], zi, -1.0, MAGICF, op0=Alu.mult, op1=Alu.add
                    )
                    r0i = npool.tile([64, 512], I32, tag="r0i", name="r0i")
                    nc.vector.tensor_copy(r0i[:], r0f[:])
                    r0 = r0i[:].bitcast(F32)
                    t_nr = npool.tile([64, 512], F32, tag="tnr", name="tnr")
                    nc.vector.tensor_tensor(t_nr[:], c[64:128, :], r0, op=Alu.mult)
                    r1 = npool.tile([64, 512], F32, tag="r1", name="r1")
                    nc.vector.scalar_tensor_tensor(
                        r1[:], t_nr[:], 2.0, r0, op0=Alu.subtract, op1=Alu.mult
                    )
                    # ctxT (negated; sign folded into wo) = ctx_u * (-1/Z)
                    nc.vector.tensor_tensor(
                        ctxT[ts(hl, 64), p, ts(qc, 512)], c[0:64, :], r1[:], op=Alu.mult
                    )

            def outproj_qt(qt):
                po = [
                    cp.tile([128, 512], F32, tag="cp", name=f"po_{d2}") for d2 in range(2)
                ]
                for hc in range(HCN):
                    for d2 in range(2):
                        nc.tensor.matmul(
                            po[d2][:],
                            lhsT=ctxT[:, hc, ts(qt, 128)],
                            rhs=wo_sb[:, hc, ts(d2, 512)],
                            start=(hc == 0),
                            stop=(hc == HCN - 1),
                        )
                for d2 in range(2):
                    o_sb = opool.tile([128, 512], BF16, tag="o", name="o_sb")
                    nc.vector.tensor_copy(o_sb[:], po[d2][:])
                    nc.sync.dma_start(out_d[qt, :, ts(d2, 512)], o_sb[:])

            # ---------------- emission schedule ----------------
            # q-proj chunk queue, pair-major: all 8 chunks of pair 0 first, etc.
            qp_queue = [(p, qp) for p in range(HCN) for qp in range(QPC)]
            # prime: k for pair 0, q for (pair0, qc0)
            for tc_i in range(4):
                kproj_chunk(0, tc_i)
            qproj_chunk(*qp_queue.pop(0))
            qproj_chunk(*qp_queue.pop(0))

            units = [(p, qc) for p in range(HCN) for qc in range(QCN)]
            prev = None
            for ui, (p, qc) in enumerate(units):
                scores_exp(p, qc)
                # fill PE with projection work (scheduler interleaves)
                if p == 0 and qc in (0, 1):
                    for sc in range(8 * qc, 8 * qc + 8):
                        vproj_chunk(sc)
                if qc < 4 and p < HCN - 1:
                    kproj_chunk(p + 1, qc)
                for _ in range(3):
                    if qp_queue:
                        qproj_chunk(*qp_queue.pop(0))
                if prev is not None:
                    pp, pqc = prev
                    ctx_unit(pp, pqc)
                    if pp == HCN - 1:
                        for qt in range(4 * pqc, 4 * pqc + 4):
                            outproj_qt(qt)
                prev = (p, qc)
            ctx_unit(*prev)
            for qt in range(12, 16):
                outproj_qt(qt)


# ---------------------------------------------------------------------------
# host side
# ---------------------------------------------------------------------------

_NC_CACHE = {}


def _get_nc(reps: int = 1):
    if reps not in _NC_CACHE:
        _NC_CACHE[reps] = build_nc(reps)
    return _NC_CACHE[reps]


def _to_bf16(a):
    return np.ascontiguousarray(np.asarray(a, np.float32)).astype(ml_dtypes.bfloat16)


def make_in_maps(q, k, v, Wq, bq, Wk, bk, Wv, bv, Wo, bo):
    """Per-core input maps (host-side sharding + layout + dtype prep)."""
    in_maps = []
    for c in range(N_CORES):
        b = c // 2
        hg = c % 2
        hs = slice(hg * NH, hg * NH + NH)

        def xt(x):
            # (S, D) -> [p, dc, t] bf16 with D = dc*128 + p
            return _to_bf16(
                np.asarray(x, np.float32).T.reshape(DC, 128, T).transpose(1, 0, 2)
            )

        def wproj(W):
            # (8, 1024, 64) -> [p, dc, hd]  (hd = h*64+dv, D = dc*128+p)
            Wc = np.asarray(W[hs], np.float32).transpose(1, 0, 2).reshape(D_MODEL, HD)
            return _to_bf16(Wc.reshape(DC, 128, HD).transpose(1, 0, 2))

        # negated + Newton-bias-corrected output weights
        wo_c = -RECIP_CORR * np.asarray(Wo[hg * HD : (hg + 1) * HD], np.float32)
        bq_c = np.asarray(bq[hs], np.float32).reshape(HD)

        in_maps.append(
            {
                "xq_t": xt(q[b]),
                "xk_t": xt(k[b]),
                "xv_t": xt(v[b]),
                "wq": wproj(Wq),
                "wk": wproj(Wk),
                "wv": wproj(Wv),
                "wo": _to_bf16(wo_c.reshape(HCN, 128, D_MODEL).transpose(1, 0, 2)),
                "bq": np.ascontiguousarray(bq_c.reshape(HCN, 128).T),
            }
        )
    return in_maps


def combine_outputs(results, bv, Wo, bo):
    """results: list of 8 dicts with 'out' (16,128,1024) bf16. Returns (B,S,D)."""
    bo_eff = np.asarray(bo, np.float32) + np.asarray(bv, np.float32).reshape(-1) @ np.asarray(
        Wo, np.float32
    )
    out = np.empty((B, S, D_MODEL), np.float32)
    for b in range(B):
        p0 = np.asarray(results[2 * b]["out"], np.float32).reshape(S, D_MODEL)
        p1 = np.asarray(results[2 * b + 1]["out"], np.float32).reshape(S, D_MODEL)
        out[b] = p0 + p1 + bo_eff
    return out


def kernel(q, k, v, Wq, bq, Wk, bk, Wv, bv, Wo, bo):
    from concourse.bass_utils import run_bass_kernel_spmd

    nc = _get_nc(1)
    in_maps = make_in_maps(q, k, v, Wq, bq, Wk, bk, Wv, bv, Wo, bo)
    res = run_bass_kernel_spmd(nc, in_maps, core_ids=list(range(N_CORES)))
    return combine_outputs(res.results, bv, Wo, bo)


# revision 9
# speedup vs baseline: 1.0938x; 1.0938x over previous
"""Multi-head attention Trainium2 kernel (v2 — pipelined single-stream).

Problem: B=4, S=2048, D_MODEL=1024, H=16 heads, d_k=d_v=64.

Sharding (8 cores, no collectives): core c handles batch b=c//2 and head
group g=c%2 (8 heads). Host sums the two head-group partials per batch and
adds the folded biases (bk drops out of softmax; bv/bo fold into bo_eff).

Engine budget per core: PE ~276us (matmuls), ACT ~255us (exp of 8*2048^2
scores, the only thing ScalarE does), DVE ~150us (psum->sbuf copies +
softmax normalization). The kernel emits one interleaved stream of
(pair, q-chunk) units so PE and ACT overlap near-fully.

Key device tricks:
 - scores^T per (pair, sc, qc): [s=128, q| two heads 512+512] psum tile via
   two concurrent K=64 row-tiled matmuls (partition halves 0-63 / 64-127).
 - exp on ScalarE only, FD=1024 per instruction.
 - ctx: K=128 (full s-chunk) matmuls, lhsT = vha[:, sc, h, 0:128] where
   cols 64:128 are ones -> psum rows 0:64 = ctx_u, rows 64:128 = Z
   replicated 64x (the PE columns were otherwise idle; N-streaming bound).
 - 1/Z on DVE WITHOUT `reciprocal` (measured ~6.4 cyc/elem): bitcast-seed
   Newton: r0 = bitcast(MAGIC - int_bits(Z)), r1 = (Z*r0 - 2)*r0 = -1/Z
   (max rel err 2.6e-3, mean -1.3e-3; sign and bias folded into Wo on
   host: wo_dev = -(1+1.29e-3) * Wo).
 - q/k/v pre-converted to bf16 on host; output partials returned as bf16.
"""

import contextlib

import numpy as np
import ml_dtypes

import concourse.bass as bass
import concourse.bacc as bacc
import concourse.mybir as mybir
import concourse.tile as tile
from concourse.bass import ts

BF16 = mybir.dt.bfloat16
F32 = mybir.dt.float32
I32 = mybir.dt.int32
Alu = mybir.AluOpType

D_MODEL, D_K, D_V, N_HEADS = 1024, 64, 64, 16
B, S = 4, 2048
N_CORES = 8
NH = 8              # heads per core
HD = NH * D_V       # 512
T = S
DC = 8              # D_MODEL / 128
SCN = 16            # s chunks of 128
QCN = 4             # q chunks of 512
HCN = 4             # head-pair chunks of 128 (= pairs)
QPC = 8             # q-proj token chunks of 256
EXP_BUFS = 18

MAGICF = float(0x7EF31999)   # Newton reciprocal seed constant
RECIP_CORR = 1.0 + 1.29e-3   # centers the Newton bias; folded into Wo


def build_nc(reps: int = 1, phases: str = "all"):
    nc = bacc.Bacc("TRN2", target_bir_lowering=False, debug=False)

    xq_d = nc.dram_tensor("xq_t", [128, DC, T], BF16, kind="ExternalInput")
    xk_d = nc.dram_tensor("xk_t", [128, DC, T], BF16, kind="ExternalInput")
    xv_d = nc.dram_tensor("xv_t", [128, DC, T], BF16, kind="ExternalInput")
    wq_d = nc.dram_tensor("wq", [128, DC, HD], BF16, kind="ExternalInput")
    wk_d = nc.dram_tensor("wk", [128, DC, HD], BF16, kind="ExternalInput")
    wv_d = nc.dram_tensor("wv", [128, DC, HD], BF16, kind="ExternalInput")
    wo_d = nc.dram_tensor("wo", [128, HCN, D_MODEL], BF16, kind="ExternalInput")
    bq_d = nc.dram_tensor("bq", [128, HCN], F32, kind="ExternalInput")
    out_d = nc.dram_tensor("out", [SCN, 128, D_MODEL], BF16, kind="ExternalOutput")

    with tile.TileContext(nc) as tc:
        def body():
            emit_body(nc, tc, xq_d, xk_d, xv_d, wq_d, wk_d, wv_d, wo_d, bq_d, out_d, phases)

        if reps == 1:
            body()
        else:
            with tc.For_i(0, reps, 1):
                body()
    nc.compile()
    return nc


def emit_body(nc, tc, xq_d, xk_d, xv_d, wq_d, wk_d, wv_d, wo_d, bq_d, out_d, phases="all"):
    ctx = contextlib.ExitStack()
    with ctx:
        # ---------------- persistent SBUF ----------------
        wpool = ctx.enter_context(tc.tile_pool(name="wpool", bufs=1))
        big = ctx.enter_context(tc.tile_pool(name="big", bufs=1))
        khp = ctx.enter_context(tc.tile_pool(name="khp", bufs=2))
        qhp = ctx.enter_context(tc.tile_pool(name="qhp", bufs=1))
        opool = ctx.enter_context(tc.tile_pool(name="opool", bufs=4))
        npool = ctx.enter_context(tc.tile_pool(name="npool", bufs=2))

        wq_sb = wpool.tile([128, DC, HD], BF16, tag="wq")
        wk_sb = wpool.tile([128, DC, HD], BF16, tag="wk")
        wv_sb = wpool.tile([128, DC, HD], BF16, tag="wv")
        wo_sb = wpool.tile([128, HCN, D_MODEL], BF16, tag="wo")
        bq_sb = wpool.tile([128, HCN], F32, tag="bq")
        xk_sb = big.tile([128, DC, T], BF16, tag="xk")

        nc.sync.dma_start(wk_sb[:], wk_d[:])
        nc.sync.dma_start(xk_sb[:], xk_d[:])
        nc.sync.dma_start(wq_sb[:], wq_d[:])
        nc.sync.dma_start(bq_sb[:], bq_d[:])
        nc.sync.dma_start(wv_sb[:], wv_d[:])
        nc.sync.dma_start(wo_sb[:], wo_d[:])

        # warm-up activation: absorbs the one-time ACT table load so the
        # first real exp (which fires ~10us in) never races the table DMA
        warm = wpool.tile([1, 16], F32, tag="warm")
        nc.vector.memset(warm[:], 0.0)
        nc.scalar.activation(warm[:], warm[:], mybir.ActivationFunctionType.Exp)
        # explicit zero bias tile for exp: avoids the implicit const-ap,
        # whose init is not ordered against early activations on first exec
        bz = wpool.tile([128, 1], F32, tag="bz")
        nc.vector.memset(bz[:], 0.0)

        qhT = qhp.tile([128, HCN, T], BF16, tag="qhT")   # [dk of 2 heads, pair, t]
        ctxT = big.tile([128, HCN, T], BF16, tag="ctxT")
        vha = big.tile([128, SCN, NH, 128], BF16, tag="vha")  # [s%128, sc, h, dv|ones]
        nc.vector.memset(vha[:, :, :, D_V:128], 1.0)

        khT = {}  # pair -> ring tile [128, T]

        with (
            tc.tile_pool(name="xst", bufs=2) as xst,
            tc.tile_pool(name="vst", bufs=2) as vst,
            tc.tile_pool(name="expool", bufs=EXP_BUFS) as expool,
            tc.tile_pool(name="sp", bufs=2, space="PSUM") as sp,
            tc.tile_pool(name="ctxp", bufs=2, space="PSUM") as ctxp,
            tc.tile_pool(name="pp", bufs=2, space="PSUM") as pp,
        ):
            exp_tiles = {}

            # ---------------- emit helpers ----------------
            def kproj_chunk(p, tc_i):
                # khT[p][:, tc-slice] = Wk[:, p-chunk].T @ xk chunk
                if p not in khT:
                    khT[p] = khp.tile([128, T], BF16, tag="khT", name=f"khT_{p}")
                pt = pp.tile([128, 512], F32, tag="pp", name=f"kp_{p}_{tc_i}")
                for dc in range(DC):
                    nc.tensor.matmul(
                        pt[:],
                        lhsT=wk_sb[:, dc, ts(p, 128)],
                        rhs=xk_sb[:, dc, ts(tc_i, 512)],
                        start=(dc == 0),
                        stop=(dc == DC - 1),
                    )
                nc.vector.tensor_copy(khT[p][:, ts(tc_i, 512)], pt[:])

            def qproj_chunk(p, qp):
                # qhT[:, p, 256-chunk qp] = Wq[:, p-chunk].T @ xq chunk + bq
                xb = xst.tile([128, DC, 256], BF16, tag="xst", name=f"xq_{p}_{qp}")
                nc.sync.dma_start(xb[:], xq_d[:, :, ts(qp, 256)])
                pt = pp.tile([128, 512], F32, tag="pp", name=f"qp_{p}_{qp}")
                for dc in range(DC):
                    nc.tensor.matmul(
                        pt[:, 0:256],
                        lhsT=wq_sb[:, dc, ts(p, 128)],
                        rhs=xb[:, dc, :],
                        start=(dc == 0),
                        stop=(dc == DC - 1),
                    )
                nc.vector.tensor_scalar_add(
                    qhT[:, p, ts(qp, 256)], pt[:, 0:256], bq_sb[:, p : p + 1]
                )

            def vproj_chunk(sc):
                # vha[:, sc, :, 0:64] = xv chunk.T @ Wv
                vx = vst.tile([128, DC, 128], BF16, tag="vst", name=f"xv_{sc}")
                nc.sync.dma_start(vx[:], xv_d[:, :, ts(sc, 128)])
                pv = pp.tile([128, 512], F32, tag="pp", name=f"vp_{sc}")
                for dc in range(DC):
                    nc.tensor.matmul(
                        pv[:],
                        lhsT=vx[:, dc, :],
                        rhs=wv_sb[:, dc, :],
                        start=(dc == 0),
                        stop=(dc == DC - 1),
                    )
                nc.vector.tensor_copy(
                    vha[:, sc, :, 0:D_V],
                    pv[:].rearrange("p (h d) -> p h d", d=D_V),
                )

            def scores_slot(p, qc, sc):
                s_ps = sp.tile([128, 1024], F32, tag="sp", name="s_ps")
                for hl in range(2):
                    pb = hl * 64
                    nc.tensor.matmul(
                        s_ps[:, ts(hl, 512)],
                        lhsT=khT[p][pb : pb + 64, ts(sc, 128)],
                        rhs=qhT[pb : pb + 64, p, ts(qc, 512)],
                        start=True,
                        stop=True,
                    )
                e = expool.tile([128, 1024], BF16, tag="exp", name="exp_t")
                nc.scalar.activation(
                    e[:], s_ps[:], mybir.ActivationFunctionType.Exp, bias=bz[:], scale=0.125
                )
                exp_tiles[(p, qc, sc)] = e

            def ctx_open():
                return [
                    ctxp.tile([128, 512], F32, tag="cp", name=f"c_{hl}") for hl in range(2)
                ]

            def ctx_mms(c_ps, p, qc, sc):
                e = exp_tiles[(p, qc, sc)]
                for hl in range(2):
                    nc.tensor.matmul(
                        c_ps[hl][:],
                        lhsT=vha[:, sc, 2 * p + hl, :],
                        rhs=e[:, ts(hl, 512)],
                        start=(sc == 0),
                        stop=(sc == SCN - 1),
                    )
                if sc == SCN - 1:
                    for s in range(SCN):
                        del exp_tiles[(p, qc, s)]

            def ctx_norm(c_ps, p, qc):
                # 1/Z via bitcast-seed + 1 Newton step (sign/bias folded into wo)
                for hl in range(2):
                    c = c_ps[hl]
                    zi = c[64:128, :].bitcast(I32)
                    r0f = npool.tile([64, 512], F32, tag="r0f", name="r0f")
                    nc.vector.tensor_scalar(
                        r0f[:], zi, -1.0, MAGICF, op0=Alu.mult, op1=Alu.add
                    )
                    r0i = npool.tile([64, 512], I32, tag="r0i", name="r0i")
                    nc.vector.tensor_copy(r0i[:], r0f[:])
                    r0 = r0i[:].bitcast(F32)
                    t_nr = npool.tile([64, 512], F32, tag="tnr", name="tnr")
                    nc.vector.tensor_tensor(t_nr[:], c[64:128, :], r0, op=Alu.mult)
                    r1 = npool.tile([64, 512], F32, tag="r1", name="r1")
                    nc.vector.scalar_tensor_tensor(
                        r1[:], t_nr[:], 2.0, r0, op0=Alu.subtract, op1=Alu.mult
                    )
                    nc.vector.tensor_tensor(
                        ctxT[ts(hl, 64), p, ts(qc, 512)], c[0:64, :], r1[:], op=Alu.mult
                    )

            def outproj_qt(qt):
                po = [
                    pp.tile([128, 512], F32, tag="pp", name=f"po_{d2}") for d2 in range(2)
                ]
                for hc in range(HCN):
                    for d2 in range(2):
                        nc.tensor.matmul(
                            po[d2][:],
                            lhsT=ctxT[:, hc, ts(qt, 128)],
                            rhs=wo_sb[:, hc, ts(d2, 512)],
                            start=(hc == 0),
                            stop=(hc == HCN - 1),
                        )
                for d2 in range(2):
                    o_sb = opool.tile([128, 512], BF16, tag="o", name="o_sb")
                    nc.vector.tensor_copy(o_sb[:], po[d2][:])
                    nc.sync.dma_start(out_d[qt, :, ts(d2, 512)], o_sb[:])

            # ---------------- emission schedule ----------------
            # One (pair, q-chunk) "unit" = 16 score slots, ACT-paced (~16us).
            # PE in-order queue: interleave everything else between score
            # slots so the PE fills the exp-drain gaps: previous unit's ctx
            # matmuls at 2 sc/step (steps 0-7), its normalization at step 8,
            # its out-projection (pair 3) after that, plus one small
            # projection piece per step from a work queue.
            qp_queue = [(p, qp) for p in range(HCN) for qp in range(QPC)]
            for tc_i in range(4):
                kproj_chunk(0, tc_i)
            qproj_chunk(*qp_queue.pop(0))
            qproj_chunk(*qp_queue.pop(0))

            units = [(p, qc) for p in range(HCN) for qc in range(QCN)]
            prev = None
            prev_c = None
            for p, qc in units:  # noqa: B007
                filler = []
                if p == 0 and qc == 0:
                    filler += [(vproj_chunk, (sc,)) for sc in range(SCN)]
                if p < HCN - 1:
                    filler.append((kproj_chunk, (p + 1, qc)))
                for _ in range(3):
                    if qp_queue:
                        filler.append((qproj_chunk, qp_queue.pop(0)))
                post = []
                if prev is not None and prev[0] == HCN - 1:
                    # out-projection of the previous q-region: must pop only
                    # AFTER ctx_norm(prev) (step 8) has emitted the ctxT write
                    post = [(outproj_qt, (qt,)) for qt in range(4 * prev[1], 4 * prev[1] + 4)]

                if prev is not None:
                    prev_c = ctx_open()
                # interleaved emission over the 16 score slots
                for sc in range(SCN):
                    scores_slot(p, qc, sc)
                    if prev is not None and sc < 8:
                        ctx_mms(prev_c, prev[0], prev[1], 2 * sc)
                        ctx_mms(prev_c, prev[0], prev[1], 2 * sc + 1)
                    if prev is not None and sc == 8:
                        ctx_norm(prev_c, prev[0], prev[1])
                    if sc >= 10 and post:
                        f, args = post.pop(0)
                        f(*args)
                    # pop filler evenly across remaining slots
                    n = (len(filler) + (SCN - 1 - sc)) // (SCN - sc)
                    for _ in range(n):
                        f, args = filler.pop(0)
                        f(*args)
                for f, args in post:
                    f(*args)
                prev = (p, qc)
            # tail: last unit's ctx + norm + final out-projection
            prev_c = ctx_open()
            for sc in range(SCN):
                ctx_mms(prev_c, prev[0], prev[1], sc)
            ctx_norm(prev_c, prev[0], prev[1])
            for qt in range(12, 16):
                outproj_qt(qt)


# ---------------------------------------------------------------------------
# host side
# ---------------------------------------------------------------------------

_NC_CACHE = {}


def _get_nc(reps: int = 1):
    if reps not in _NC_CACHE:
        _NC_CACHE[reps] = build_nc(reps)
    return _NC_CACHE[reps]


def _to_bf16(a):
    return np.ascontiguousarray(np.asarray(a, np.float32)).astype(ml_dtypes.bfloat16)


def make_in_maps(q, k, v, Wq, bq, Wk, bk, Wv, bv, Wo, bo):
    """Per-core input maps (host-side sharding + layout + dtype prep)."""
    in_maps = []
    for c in range(N_CORES):
        b = c // 2
        hg = c % 2
        hs = slice(hg * NH, hg * NH + NH)

        def xt(x):
            # (S, D) -> [p, dc, t] bf16 with D = dc*128 + p
            return _to_bf16(
                np.asarray(x, np.float32).T.reshape(DC, 128, T).transpose(1, 0, 2)
            )

        def wproj(W):
            # (8, 1024, 64) -> [p, dc, hd]  (hd = h*64+dv, D = dc*128+p)
            Wc = np.asarray(W[hs], np.float32).transpose(1, 0, 2).reshape(D_MODEL, HD)
            return _to_bf16(Wc.reshape(DC, 128, HD).transpose(1, 0, 2))

        # negated + Newton-bias-corrected output weights
        wo_c = -RECIP_CORR * np.asarray(Wo[hg * HD : (hg + 1) * HD], np.float32)
        bq_c = np.asarray(bq[hs], np.float32).reshape(HD)

        in_maps.append(
            {
                "xq_t": xt(q[b]),
                "xk_t": xt(k[b]),
                "xv_t": xt(v[b]),
                "wq": wproj(Wq),
                "wk": wproj(Wk),
                "wv": wproj(Wv),
                "wo": _to_bf16(wo_c.reshape(HCN, 128, D_MODEL).transpose(1, 0, 2)),
                "bq": np.ascontiguousarray(bq_c.reshape(HCN, 128).T),
            }
        )
    return in_maps


def combine_outputs(results, bv, Wo, bo):
    """results: list of 8 dicts with 'out' (16,128,1024) bf16. Returns (B,S,D)."""
    bo_eff = np.asarray(bo, np.float32) + np.asarray(bv, np.float32).reshape(-1) @ np.asarray(
        Wo, np.float32
    )
    out = np.empty((B, S, D_MODEL), np.float32)
    for b in range(B):
        p0 = np.asarray(results[2 * b]["out"], np.float32).reshape(S, D_MODEL)
        p1 = np.asarray(results[2 * b + 1]["out"], np.float32).reshape(S, D_MODEL)
        out[b] = p0 + p1 + bo_eff
    return out


def kernel(q, k, v, Wq, bq, Wk, bk, Wv, bv, Wo, bo):
    from concourse.bass_utils import run_bass_kernel_spmd

    nc = _get_nc(1)
    in_maps = make_in_maps(q, k, v, Wq, bq, Wk, bk, Wv, bv, Wo, bo)
    res = run_bass_kernel_spmd(nc, in_maps, core_ids=list(range(N_CORES)))
    return combine_outputs(res.results, bv, Wo, bo)
